# revision 4
# baseline (speedup 1.0000x reference)
"""Gcs pairwise-distance loss kernel for Trainium2 (Bass/Tile), 8-core SPMD.

Math: with d = pred - truth, dX = d[:, :P], dY = d[:, P:] (B=32, P=1024),
    sumsq_h[i] = sum_{b,j} (v[b,j] - v[b,i])^2
               = S2_h + sum_b (1024*v[b,i]^2 - 2*rs_h[b]*v[b,i])
where rs_h[b] = sum_j v[b,j], S2_h = sum_{b,j} v[b,j]^2.  The loss is
    (sum_i sqrt(sumsq_X[i]) + sum_i sqrt(sumsq_Y[i])) / 64.
This collapses the O(B*P^2) pairwise reduction to O(B*P).

Layout: d [32, 2048] is viewed as [128, 512]; partition p = 4*b + c where
c in {0,1} covers X columns and {2,3} covers Y columns.

v2 schedule (vs the fp32 baseline, from neuron-profile traces):
- inputs are cast to bf16 on the host (tolerance is 2e-2; the cast costs
  ~0.3% worst case): halves DMA bytes and doubles DVE throughput.
- two input DMAs instead of four: in0 = [pred_h0 | truth_h0], in1 =
  [pred_h1 | truth_h1], so each HW queue (sync Q1 / scalar Q10) carries
  exactly what one subtract consumes.
- consts ride the same two HW queues behind the inputs (no slow gpsimd
  software queue), shrunk from 102KB to ~36KB.
- the row sums fold into the subtract STTs via accum_out (the baseline
  spent 2x419ns on standalone tensor_reduce).
- the whole element path (td/dsq/comb) is bf16, so the main matmul is a
  single-pass bf16 matmul instead of two fp32 LOW/HIGH passes.
- the comb STT reads the per-partition hsm2 scalar straight from PSUM.

Every core computes the full replicated result (inputs are only 256KB,
far below the ~20us collective all-reduce floor, so replication beats
batch-sharding + AllReduce); core 0's scalar is returned.
"""

import numpy as np

_CACHE = {}


def _build_consts():
    # bf16 [128, 132]:
    #   cols 0:128   hconst[k,m] = -2 if k//2==m//2   (lhsT, pair-sum matmul)
    #   cols 128:132 mask4[p,m]  = 1 if p%4==m        (lhsT, main matmul)
    # f32 [128, 4]:
    #   maskS4[p,m] = 1/(1024*4096) if (p%4)//2==m//2 (lhsT, S2 matmul)
    import ml_dtypes

    cb = np.zeros((128, 132), dtype=np.float32)
    k = np.arange(128)
    for m in range(128):
        cb[k[k // 2 == m // 2], m] = -2.0
    for m in range(4):
        cb[k[k % 4 == m], 128 + m] = 1.0
    cf = np.zeros((128, 4), dtype=np.float32)
    for m in range(4):
        cf[k[(k % 4) // 2 == m // 2], m] = 1.0 / 1024.0 / 4096.0
    return cb.astype(ml_dtypes.bfloat16), cf


def _build_nc():
    import concourse.tile as tile
    from concourse import bacc, mybir

    f32 = mybir.dt.float32
    bf16 = mybir.dt.bfloat16
    nc = bacc.Bacc("TRN2", target_bir_lowering=False, debug=False)
    in0 = nc.dram_tensor("in0", [128, 512], bf16, kind="ExternalInput").ap()
    in1 = nc.dram_tensor("in1", [128, 512], bf16, kind="ExternalInput").ap()
    cb = nc.dram_tensor("cb", [128, 132], bf16, kind="ExternalInput").ap()
    cf = nc.dram_tensor("cf", [128, 4], f32, kind="ExternalInput").ap()
    out = nc.dram_tensor("out", [1, 1], f32, kind="ExternalOutput").ap()

    H = 256

    with tile.TileContext(nc) as tc:
        with (
            tc.tile_pool(name="sb", bufs=1) as sb,
            tc.tile_pool(name="ps", bufs=1, space="PSUM") as ps,
        ):
            tin0 = sb.tile([128, 512], bf16, tag="tin0")
            tin1 = sb.tile([128, 512], bf16, tag="tin1")
            tcb = sb.tile([128, 132], bf16, tag="tcb")
            tcf = sb.tile([128, 4], f32, tag="tcf")
            # inputs first on each queue, consts behind them (FIFO per
            # queue: consts land ~0.3us after the input they trail).
            nc.sync.dma_start(tin0[:, :], in0)
            nc.scalar.dma_start(tin1[:, :], in1)
            nc.sync.dma_start(tcb[:, :], cb)
            nc.scalar.dma_start(tcf[:, :], cf)
            hconst = tcb[:, 0:128]
            mask4 = tcb[:, 128:132]
            maskS4 = tcf[:, 0:4]

            # Dummy sqrt on a 1x1 scratch: forces the sqrt act-table load
            # (set 3, which also holds Square) to issue right after the DMA
            # issues.  Table loads are non-blocking descriptor issues, so
            # this hides the 1.28us table DMA under the input transfers
            # instead of letting it gate the final Sqrt.
            scratch = sb.tile([1, 1], f32, tag="scratch")
            dummy = sb.tile([1, 1], f32, tag="dummy")
            with tc.high_priority():
                nc.gpsimd.memset(scratch[:, :], 1.0)
                nc.scalar.activation(
                    dummy[:, :], scratch[:, :],
                    mybir.ActivationFunctionType.Sqrt,
                )

            td = sb.tile([128, 512], bf16, tag="td")
            dsq = sb.tile([128, 512], bf16, tag="dsq")
            red0 = sb.tile([128, 1], f32, tag="red0")
            red1 = sb.tile([128, 1], f32, tag="red1")
            acc = sb.tile([128, 1], f32, tag="acc")

            # d = pred - truth with the row sum fused in via accum_out
            nc.vector.scalar_tensor_tensor(
                out=td[:, 0:H], in0=tin0[:, 0:H], scalar=1.0,
                in1=tin0[:, H:512],
                op0=mybir.AluOpType.mult, op1=mybir.AluOpType.subtract,
                accum_out=red0[:, :],
            )
            nc.vector.scalar_tensor_tensor(
                out=td[:, H:512], in0=tin1[:, 0:H], scalar=1.0,
                in1=tin1[:, H:512],
                op0=mybir.AluOpType.mult, op1=mybir.AluOpType.subtract,
                accum_out=red1[:, :],
            )
            # cs_db = red0 + red1 (bf16) unblocks the PE pair-sum matmul
            cs_db = sb.tile([128, 1], bf16, tag="cs_db")
            with tc.high_priority():
                nc.vector.tensor_add(cs_db[:, :], red0[:, :], red1[:, :])
            # dsq = (32*d)^2 = 1024*d^2 on the otherwise-idle Scalar engine
            # (every act table holds Square), acc = sum_j 1024*d^2.  This
            # takes the square off the serial DVE chain.
            nc.scalar.activation(
                dsq[:, :], td[:, :], mybir.ActivationFunctionType.Square,
                scale=32.0, accum_out=acc[:, :],
            )

            # hsm2[p] = -2*(cs_d[p] + cs_d[p^1])  (bf16 single-pass matmul)
            hsm2 = ps.tile([128, 1], f32, tag="hsm2")
            nc.tensor.matmul(hsm2[:, :], hconst, cs_db[:, :], start=True, stop=True)

            # S2 per output row (fp32, tiny N=1; runs in the PE gap)
            s2 = ps.tile([4, 1], f32, tag="s2")
            nc.tensor.matmul(s2[:, :], maskS4, acc[:, :], start=True, stop=True)

            # comb = d*hsm2 + 1024*d^2; hsm2 scalar read straight from PSUM
            comb = sb.tile([128, 512], bf16, tag="comb")
            nc.vector.scalar_tensor_tensor(
                out=comb[:, :], in0=td[:, :], scalar=hsm2[:, :],
                in1=dsq[:, :],
                op0=mybir.AluOpType.mult, op1=mybir.AluOpType.add,
            )
            main = ps.tile([4, 512], f32, tag="main")
            nc.tensor.matmul(main[:, :], mask4, comb[:, :], start=True, stop=True)

            bias = sb.tile([4, 1], f32, tag="bias")
            nc.vector.tensor_copy(bias[:, :], s2[:, :])

            # dist = sqrt(main/4096 + bias); dsums[m] = sum_j dist[m,j]
            # scale=2^-12 folds the /64 into the sqrt: sqrt(x/4096)=sqrt(x)/64
            dist = sb.tile([4, 512], f32, tag="dist")
            dsums = sb.tile([4, 1], f32, tag="dsums")
            nc.scalar.activation(
                dist[:, :], main[:, :], mybir.ActivationFunctionType.Sqrt,
                bias=bias[:, :], scale=1.0 / 4096.0, accum_out=dsums[:, :],
            )

            # total = sum_m dsums[m]  (4-partition sum on gpsimd)
            out_sb = sb.tile([1, 1], f32, tag="out_sb")
            nc.gpsimd.tensor_reduce(
                out=out_sb[:, :], in_=dsums[:, :], axis=mybir.AxisListType.C,
                op=mybir.AluOpType.add,
            )
            nc.sync.dma_start(out, out_sb[:, :])

    nc.compile()
    return nc


def _get():
    if "nc" not in _CACHE:
        _CACHE["nc"] = _build_nc()
        _CACHE["cb"], _CACHE["cf"] = _build_consts()
    return _CACHE["nc"]


def _in_map(pred, truth):
    import ml_dtypes

    nc = _get()
    bf16 = ml_dtypes.bfloat16
    p = np.ascontiguousarray(np.asarray(pred, dtype=np.float32)).reshape(128, 512)
    t = np.ascontiguousarray(np.asarray(truth, dtype=np.float32)).reshape(128, 512)
    H = 256
    in0 = np.empty((128, 512), dtype=bf16)
    in1 = np.empty((128, 512), dtype=bf16)
    in0[:, 0:H] = p[:, 0:H].astype(bf16)
    in0[:, H:512] = t[:, 0:H].astype(bf16)
    in1[:, 0:H] = p[:, H:512].astype(bf16)
    in1[:, H:512] = t[:, H:512].astype(bf16)
    return nc, {"in0": in0, "in1": in1, "cb": _CACHE["cb"], "cf": _CACHE["cf"]}


def kernel(pred, truth) -> np.ndarray:
    from concourse.bass_utils import run_bass_kernel_spmd

    nc, in_map = _in_map(pred, truth)
    res = run_bass_kernel_spmd(
        nc, [dict(in_map) for _ in range(8)], core_ids=list(range(8))
    )
    return res.results[0]["out"].reshape(()).astype(np.float32)


# revision 5
# speedup vs baseline: 1.1539x; 1.1539x over previous
"""Gcs pairwise-distance loss kernel for Trainium2 (Bass/Tile), 8-core SPMD.

Math: with d = pred - truth, dX = d[:, :P], dY = d[:, P:] (B=32, P=1024),
    sumsq_h[i] = sum_{b,j} (v[b,j] - v[b,i])^2
               = S2_h + sum_b (1024*v[b,i]^2 - 2*rs_h[b]*v[b,i])
where rs_h[b] = sum_j v[b,j], S2_h = sum_{b,j} v[b,j]^2.  The loss is
    (sum_i sqrt(sumsq_X[i]) + sum_i sqrt(sumsq_Y[i])) / 64.
This collapses the O(B*P^2) pairwise reduction to O(B*P).

Layout: d [32, 2048] is viewed as [128, 512]; partition p = 4*b + c where
c in {0,1} covers X columns and {2,3} covers Y columns.

v4 schedule (traced against v2/v3 on neuron-profile):
- inputs are cast to fp8-e4m3 on the host (loss is a 65k-term reduction:
  quantization error averages out, measured ~1e-3 rel vs the 2e-2 gate):
  64KB per HW queue, landing ~0.5us earlier than bf16.
- a dummy 1x1 sqrt right after the DMA issues forces the sqrt act-table
  (set 3, which also holds Square) to load under the input transfers;
  act-table loads are non-blocking descriptor issues.
- d = pred - truth on DVE with the row sums fused in via accum_out.
- dsq = (32d)^2 runs on the otherwise-idle Scalar engine in two column
  halves so comb/mm pipeline behind it.
- comb = d*hsm2 + dsq on DVE (hsm2 scalar read straight from PSUM),
  main matmul in bf16 single pass, both in column halves.
- S2 matmul in bf16 (acc converted once) so the sqrt bias is ready
  before the second main matmul finishes.

Every core computes the full replicated result (inputs are tiny, far
below the ~20us collective floor, so replication beats batch-sharding +
AllReduce); core 0's scalar is returned.
"""

import numpy as np

_CACHE = {}


def _build_consts():
    # bf16 [128, 136]:
    #   cols 0:128   hconst[k,m] = -2 if k//2==m//2   (lhsT, pair-sum matmul)
    #   cols 128:132 mask4[p,m]  = 1 if p%4==m        (lhsT, main matmul)
    #   cols 132:136 maskS4[p,m] = 2^-22 if (p%4)//2==m//2 (lhsT, S2 matmul)
    import ml_dtypes

    cb = np.zeros((128, 136), dtype=np.float32)
    k = np.arange(128)
    for m in range(128):
        cb[k[k // 2 == m // 2], m] = -2.0
    for m in range(4):
        cb[k[k % 4 == m], 128 + m] = 1.0
        cb[k[(k % 4) // 2 == m // 2], 132 + m] = 1.0 / 1024.0 / 4096.0
    return cb.astype(ml_dtypes.bfloat16)


def _build_nc():
    import concourse.tile as tile
    from concourse import bacc, mybir

    f32 = mybir.dt.float32
    bf16 = mybir.dt.bfloat16
    fp8 = mybir.dt.float8e4
    nc = bacc.Bacc("TRN2", target_bir_lowering=False, debug=False)
    in0 = nc.dram_tensor("in0", [128, 512], fp8, kind="ExternalInput").ap()
    in1 = nc.dram_tensor("in1", [128, 512], fp8, kind="ExternalInput").ap()
    cb = nc.dram_tensor("cb", [128, 136], bf16, kind="ExternalInput").ap()
    out = nc.dram_tensor("out", [1, 1], f32, kind="ExternalOutput").ap()

    H = 256

    with tile.TileContext(nc) as tc:
        with (
            tc.tile_pool(name="sb", bufs=1) as sb,
            tc.tile_pool(name="ps", bufs=1, space="PSUM") as ps,
        ):
            tin0 = sb.tile([128, 512], fp8, tag="tin0")
            tin1 = sb.tile([128, 512], fp8, tag="tin1")
            tcb = sb.tile([128, 136], bf16, tag="tcb")
            nc.sync.dma_start(tin0[:, :], in0)
            nc.scalar.dma_start(tin1[:, :], in1)
            nc.sync.dma_start(tcb[:, :], cb)
            hconst = tcb[:, 0:128]
            mask4 = tcb[:, 128:132]
            maskS4 = tcb[:, 132:136]

            # Dummy sqrt on a 1x1 scratch: forces the sqrt act-table load
            # (set 3, which also holds Square) to issue right after the DMA
            # issues instead of gating the final Sqrt.
            scratch = sb.tile([1, 1], f32, tag="scratch")
            dummy = sb.tile([1, 1], f32, tag="dummy")
            with tc.high_priority():
                nc.gpsimd.memset(scratch[:, :], 1.0)
                nc.scalar.activation(
                    dummy[:, :], scratch[:, :],
                    mybir.ActivationFunctionType.Sqrt,
                )

            td = sb.tile([128, 512], bf16, tag="td")
            dsq = sb.tile([128, 512], bf16, tag="dsq")
            red0 = sb.tile([128, 1], f32, tag="red0")
            red1 = sb.tile([128, 1], f32, tag="red1")
            acc_a = sb.tile([128, 1], f32, tag="acc_a")
            acc_b = sb.tile([128, 1], f32, tag="acc_b")

            # d = pred - truth with the row sum fused in via accum_out
            nc.vector.scalar_tensor_tensor(
                out=td[:, 0:H], in0=tin0[:, 0:H], scalar=1.0,
                in1=tin0[:, H:512],
                op0=mybir.AluOpType.mult, op1=mybir.AluOpType.subtract,
                accum_out=red0[:, :],
            )
            nc.vector.scalar_tensor_tensor(
                out=td[:, H:512], in0=tin1[:, 0:H], scalar=1.0,
                in1=tin1[:, H:512],
                op0=mybir.AluOpType.mult, op1=mybir.AluOpType.subtract,
                accum_out=red1[:, :],
            )
            # cs_db = red0 + red1 (bf16) unblocks the PE pair-sum matmul
            cs_db = sb.tile([128, 1], bf16, tag="cs_db")
            with tc.high_priority():
                nc.vector.tensor_add(cs_db[:, :], red0[:, :], red1[:, :])

            # hsm2[p] = -2*(cs_d[p] + cs_d[p^1])  (bf16 single-pass matmul)
            hsm2 = ps.tile([128, 1], f32, tag="hsm2")
            nc.tensor.matmul(hsm2[:, :], hconst, cs_db[:, :], start=True, stop=True)

            # dsq = (32*d)^2 = 1024*d^2 on the Scalar engine, in halves so
            # comb/mm pipeline behind it; acc_* catch the per-half row sums
            nc.scalar.activation(
                dsq[:, 0:H], td[:, 0:H], mybir.ActivationFunctionType.Square,
                scale=32.0, accum_out=acc_a[:, :],
            )
            nc.scalar.activation(
                dsq[:, H:512], td[:, H:512],
                mybir.ActivationFunctionType.Square,
                scale=32.0, accum_out=acc_b[:, :],
            )

            # comb = d*hsm2 + 1024*d^2; hsm2 scalar read straight from PSUM
            comb = sb.tile([128, 512], bf16, tag="comb")
            main = ps.tile([4, 512], f32, tag="main")
            nc.vector.scalar_tensor_tensor(
                out=comb[:, 0:H], in0=td[:, 0:H], scalar=hsm2[:, :],
                in1=dsq[:, 0:H],
                op0=mybir.AluOpType.mult, op1=mybir.AluOpType.add,
            )
            nc.tensor.matmul(main[:, 0:H], mask4, comb[:, 0:H], start=True, stop=True)
            nc.vector.scalar_tensor_tensor(
                out=comb[:, H:512], in0=td[:, H:512], scalar=hsm2[:, :],
                in1=dsq[:, H:512],
                op0=mybir.AluOpType.mult, op1=mybir.AluOpType.add,
            )
            nc.tensor.matmul(main[:, H:512], mask4, comb[:, H:512], start=True, stop=True)

            # acc in bf16 so the S2 matmul is a single bf16 pass
            acc_bf = sb.tile([128, 1], bf16, tag="acc_bf")
            nc.vector.tensor_add(acc_bf[:, :], acc_a[:, :], acc_b[:, :])
            s2 = ps.tile([4, 1], f32, tag="s2")
            nc.tensor.matmul(s2[:, :], maskS4, acc_bf[:, :], start=True, stop=True)
            bias = sb.tile([4, 1], f32, tag="bias")
            nc.vector.tensor_copy(bias[:, :], s2[:, :])

            # dist = sqrt(main/4096 + bias); dsums[m] = sum_j dist[m,j]
            # scale=2^-12 folds the /64 into the sqrt: sqrt(x/4096)=sqrt(x)/64
            dist = sb.tile([4, 512], f32, tag="dist")
            dsums = sb.tile([4, 1], f32, tag="dsums")
            nc.scalar.activation(
                dist[:, :], main[:, :], mybir.ActivationFunctionType.Sqrt,
                bias=bias[:, :], scale=1.0 / 4096.0, accum_out=dsums[:, :],
            )

            # total = sum_m dsums[m]  (4-partition sum on gpsimd)
            out_sb = sb.tile([1, 1], f32, tag="out_sb")
            nc.gpsimd.tensor_reduce(
                out=out_sb[:, :], in_=dsums[:, :], axis=mybir.AxisListType.C,
                op=mybir.AluOpType.add,
            )
            nc.sync.dma_start(out, out_sb[:, :])

    nc.compile()
    return nc


def _get():
    if "nc" not in _CACHE:
        _CACHE["nc"] = _build_nc()
        _CACHE["cb"] = _build_consts()
    return _CACHE["nc"]


def _in_map(pred, truth):
    import ml_dtypes

    nc = _get()
    fp8 = ml_dtypes.float8_e4m3
    p = np.ascontiguousarray(np.asarray(pred, dtype=np.float32)).reshape(128, 512)
    t = np.ascontiguousarray(np.asarray(truth, dtype=np.float32)).reshape(128, 512)
    H = 256
    in0 = np.empty((128, 512), dtype=fp8)
    in1 = np.empty((128, 512), dtype=fp8)
    in0[:, 0:H] = p[:, 0:H].astype(fp8)
    in0[:, H:512] = t[:, 0:H].astype(fp8)
    in1[:, 0:H] = p[:, H:512].astype(fp8)
    in1[:, H:512] = t[:, H:512].astype(fp8)
    return nc, {"in0": in0, "in1": in1, "cb": _CACHE["cb"]}


def kernel(pred, truth) -> np.ndarray:
    from concourse.bass_utils import run_bass_kernel_spmd

    nc, in_map = _in_map(pred, truth)
    res = run_bass_kernel_spmd(
        nc, [dict(in_map) for _ in range(8)], core_ids=list(range(8))
    )
    return res.results[0]["out"].reshape(()).astype(np.float32)


# revision 9
# speedup vs baseline: 1.2217x; 1.0588x over previous
"""Gcs pairwise-distance loss kernel for Trainium2 (Bass/Tile), 8-core SPMD.

Math: with d = pred - truth, dX = d[:, :P], dY = d[:, P:] (B=32, P=1024),
    sumsq_h[i] = sum_{b,j} (v[b,j] - v[b,i])^2
               = S2_h + sum_b (1024*v[b,i]^2 - 2*rs_h[b]*v[b,i])
where rs_h[b] = sum_j v[b,j], S2_h = sum_{b,j} v[b,j]^2.  The loss is
    (sum_i sqrt(sumsq_X[i]) + sum_i sqrt(sumsq_Y[i])) / 64.
This collapses the O(B*P^2) pairwise reduction to O(B*P).

Layout: d [32, 2048] is viewed as [128, 512]; partition p = 4*b + c where
c in {0,1} covers X columns and {2,3} covers Y columns.

v4 schedule (traced against v2/v3 on neuron-profile):
- inputs are cast to fp8-e4m3 on the host (loss is a 65k-term reduction:
  quantization error averages out, measured ~1e-3 rel vs the 2e-2 gate):
  64KB per HW queue, landing ~0.5us earlier than bf16.
- a dummy 1x1 sqrt right after the DMA issues forces the sqrt act-table
  (set 3, which also holds Square) to load under the input transfers;
  act-table loads are non-blocking descriptor issues.
- d = pred - truth on DVE with the row sums fused in via accum_out.
- dsq = (32d)^2 runs on the otherwise-idle Scalar engine in two column
  halves so comb/mm pipeline behind it.
- comb = d*hsm2 + dsq on DVE (hsm2 scalar read straight from PSUM),
  main matmul in bf16 single pass, both in column halves.
- S2 matmul in bf16 (acc converted once) so the sqrt bias is ready
  before the second main matmul finishes.

Every core computes the full replicated result (inputs are tiny, far
below the ~20us collective floor, so replication beats batch-sharding +
AllReduce); core 0's scalar is returned.
"""

import numpy as np

_CACHE = {}


def _build_consts():
    # bf16 [128, 152]:
    #   cols 0:128   hconst[k,m] = -2 if k//2==m//2   (lhsT, pair-sum matmul)
    #   cols 128:136 mask8a[p,m] = 1 if m<4 and p%4==m    (lhsT, main mm half a)
    #   cols 136:144 mask8b[p,m] = 1 if m>=4 and p%4==m-4 (lhsT, main mm half b)
    #   cols 144:152 maskS8[p,m] = 2^-22 if (p%4)//2==(m%4)//2 (lhsT, S2 matmul)
    # The main matmul writes an [8, 256] PSUM tile (rows 0:4 = column half a,
    # rows 4:8 = half b) so the final Sqrt runs 256 long on 8 partitions.
    import ml_dtypes

    cb = np.zeros((128, 152), dtype=np.float32)
    k = np.arange(128)
    for m in range(128):
        cb[k[k // 2 == m // 2], m] = -2.0
    for m in range(4):
        cb[k[k % 4 == m], 128 + m] = 1.0
        cb[k[k % 4 == m], 140 + m] = 1.0
    for m in range(8):
        cb[k[(k % 4) // 2 == (m % 4) // 2], 144 + m] = 1.0 / 1024.0 / 4096.0
    return cb.astype(ml_dtypes.bfloat16)


def _build_nc():
    import concourse.tile as tile
    from concourse import bacc, mybir

    f32 = mybir.dt.float32
    bf16 = mybir.dt.bfloat16
    fp8 = mybir.dt.float8e4
    nc = bacc.Bacc("TRN2", target_bir_lowering=False, debug=False)
    in0 = nc.dram_tensor("in0", [128, 512], fp8, kind="ExternalInput").ap()
    in1 = nc.dram_tensor("in1", [128, 512], fp8, kind="ExternalInput").ap()
    cb = nc.dram_tensor("cb", [128, 152], bf16, kind="ExternalInput").ap()
    out = nc.dram_tensor("out", [1, 1], f32, kind="ExternalOutput").ap()

    H = 256

    with tile.TileContext(nc) as tc:
        with (
            tc.tile_pool(name="sb", bufs=1) as sb,
            tc.tile_pool(name="ps", bufs=1, space="PSUM") as ps,
        ):
            tin0 = sb.tile([128, 512], fp8, tag="tin0")
            tin1 = sb.tile([128, 512], fp8, tag="tin1")
            tcb = sb.tile([128, 152], bf16, tag="tcb")
            nc.sync.dma_start(tin0[:, :], in0)
            nc.scalar.dma_start(tin1[:, :], in1)
            nc.sync.dma_start(tcb[:, :], cb)
            hconst = tcb[:, 0:128]
            mask8a = tcb[:, 128:136]
            mask8b = tcb[:, 136:144]
            maskS8 = tcb[:, 144:152]

            # Dummy sqrt on a 1x1 scratch: forces the sqrt act-table load
            # (set 3, which also holds Square) to issue right after the DMA
            # issues instead of gating the final Sqrt.
            scratch = sb.tile([1, 1], f32, tag="scratch")
            dummy = sb.tile([1, 1], f32, tag="dummy")
            with tc.high_priority():
                nc.gpsimd.memset(scratch[:, :], 1.0)
                nc.scalar.activation(
                    dummy[:, :], scratch[:, :],
                    mybir.ActivationFunctionType.Sqrt,
                )

            td = sb.tile([128, 512], bf16, tag="td")
            dsq = sb.tile([128, 512], bf16, tag="dsq")
            red0 = sb.tile([128, 1], f32, tag="red0")
            red1 = sb.tile([128, 1], f32, tag="red1")
            acc_a = sb.tile([128, 1], f32, tag="acc_a")
            acc_b = sb.tile([128, 1], f32, tag="acc_b")

            # d = pred - truth with the row sum fused in via accum_out
            nc.vector.scalar_tensor_tensor(
                out=td[:, 0:H], in0=tin0[:, 0:H], scalar=1.0,
                in1=tin0[:, H:512],
                op0=mybir.AluOpType.mult, op1=mybir.AluOpType.subtract,
                accum_out=red0[:, :],
            )
            nc.vector.scalar_tensor_tensor(
                out=td[:, H:512], in0=tin1[:, 0:H], scalar=1.0,
                in1=tin1[:, H:512],
                op0=mybir.AluOpType.mult, op1=mybir.AluOpType.subtract,
                accum_out=red1[:, :],
            )
            # cs_db = red0 + red1 (bf16) unblocks the PE pair-sum matmul
            cs_db = sb.tile([128, 1], bf16, tag="cs_db")
            with tc.high_priority():
                nc.vector.tensor_add(cs_db[:, :], red0[:, :], red1[:, :])

            # hsm2[p] = -2*(cs_d[p] + cs_d[p^1])  (bf16 single-pass matmul)
            hsm2 = ps.tile([128, 1], f32, tag="hsm2")
            nc.tensor.matmul(hsm2[:, :], hconst, cs_db[:, :], start=True, stop=True)

            # dsq = (32*d)^2 = 1024*d^2 on the Scalar engine, in halves so
            # comb/mm pipeline behind it; acc_* catch the per-half row sums
            nc.scalar.activation(
                dsq[:, 0:H], td[:, 0:H], mybir.ActivationFunctionType.Square,
                scale=32.0, accum_out=acc_a[:, :],
            )
            nc.scalar.activation(
                dsq[:, H:512], td[:, H:512],
                mybir.ActivationFunctionType.Square,
                scale=32.0, accum_out=acc_b[:, :],
            )

            # comb = d*hsm2 + 1024*d^2; hsm2 scalar read straight from PSUM.
            # The two column halves land on PSUM rows 0:4 / 4:8 of one [8,256]
            # accumulation group so the final Sqrt is 256 long on 8 partitions.
            comb = sb.tile([128, 512], bf16, tag="comb")
            main = ps.tile([8, H], f32, tag="main")
            nc.vector.scalar_tensor_tensor(
                out=comb[:, 0:H], in0=td[:, 0:H], scalar=hsm2[:, :],
                in1=dsq[:, 0:H],
                op0=mybir.AluOpType.mult, op1=mybir.AluOpType.add,
            )
            nc.tensor.matmul(main[:, :], mask8a, comb[:, 0:H], start=True, stop=False)
            # acc in bf16 so the S2 matmul is a single bf16 pass; emitted
            # before comb_b so s2/bias are ready ahead of the last matmul
            acc_bf = sb.tile([128, 1], bf16, tag="acc_bf")
            nc.vector.tensor_add(acc_bf[:, :], acc_a[:, :], acc_b[:, :])
            s2 = ps.tile([8, 1], f32, tag="s2")
            nc.tensor.matmul(s2[:, :], maskS8, acc_bf[:, :], start=True, stop=True)
            bias = sb.tile([8, 1], f32, tag="bias")
            nc.vector.tensor_copy(bias[:, :], s2[:, :])
            nc.vector.scalar_tensor_tensor(
                out=comb[:, H:512], in0=td[:, H:512], scalar=hsm2[:, :],
                in1=dsq[:, H:512],
                op0=mybir.AluOpType.mult, op1=mybir.AluOpType.add,
            )
            nc.tensor.matmul(main[:, :], mask8b, comb[:, H:512], start=False, stop=True)

            # dist = sqrt(main/4096 + bias); dsums[m] = sum_j dist[m,j]
            # scale=2^-12 folds the /64 into the sqrt: sqrt(x/4096)=sqrt(x)/64
            dist = sb.tile([8, H], f32, tag="dist")
            dsums = sb.tile([8, 1], f32, tag="dsums")
            nc.scalar.activation(
                dist[:, :], main[:, :], mybir.ActivationFunctionType.Sqrt,
                bias=bias[:, :], scale=1.0 / 4096.0, accum_out=dsums[:, :],
            )

            # total = sum_m dsums[m]  (8-partition sum on gpsimd)
            out_sb = sb.tile([1, 1], f32, tag="out_sb")
            nc.gpsimd.tensor_reduce(
                out=out_sb[:, :], in_=dsums[:, :], axis=mybir.AxisListType.C,
                op=mybir.AluOpType.add,
            )
            nc.sync.dma_start(out, out_sb[:, :])

    nc.compile()
    return nc


def _get():
    if "nc" not in _CACHE:
        _CACHE["nc"] = _build_nc()
        _CACHE["cb"] = _build_consts()
    return _CACHE["nc"]


def _in_map(pred, truth):
    import ml_dtypes

    nc = _get()
    fp8 = ml_dtypes.float8_e4m3
    p = np.ascontiguousarray(np.asarray(pred, dtype=np.float32)).reshape(128, 512)
    t = np.ascontiguousarray(np.asarray(truth, dtype=np.float32)).reshape(128, 512)
    H = 256
    in0 = np.empty((128, 512), dtype=fp8)
    in1 = np.empty((128, 512), dtype=fp8)
    in0[:, 0:H] = p[:, 0:H].astype(fp8)
    in0[:, H:512] = t[:, 0:H].astype(fp8)
    in1[:, 0:H] = p[:, H:512].astype(fp8)
    in1[:, H:512] = t[:, H:512].astype(fp8)
    return nc, {"in0": in0, "in1": in1, "cb": _CACHE["cb"]}


def kernel(pred, truth) -> np.ndarray:
    from concourse.bass_utils import run_bass_kernel_spmd

    nc, in_map = _in_map(pred, truth)
    res = run_bass_kernel_spmd(
        nc, [dict(in_map) for _ in range(8)], core_ids=list(range(8))
    )
    return res.results[0]["out"].reshape(()).astype(np.float32)


# revision 15
# speedup vs baseline: 1.2250x; 1.0027x over previous
"""Gcs pairwise-distance loss kernel for Trainium2 (Bass/Tile), 8-core SPMD.

Math: with d = pred - truth, dX = d[:, :P], dY = d[:, P:] (B=32, P=1024),
    sumsq_h[i] = sum_{b,j} (v[b,j] - v[b,i])^2
               = S2_h + sum_b (1024*v[b,i]^2 - 2*rs_h[b]*v[b,i])
where rs_h[b] = sum_j v[b,j], S2_h = sum_{b,j} v[b,j]^2.  The loss is
    (sum_i sqrt(sumsq_X[i]) + sum_i sqrt(sumsq_Y[i])) / 64.
This collapses the O(B*P^2) pairwise reduction to O(B*P).

Layout: d [32, 2048] is viewed as [128, 512]; partition p = 4*b + c where
c in {0,1} covers X columns and {2,3} covers Y columns.

v4 schedule (traced against v2/v3 on neuron-profile):
- inputs are cast to fp8-e4m3 on the host (loss is a 65k-term reduction:
  quantization error averages out, measured ~1e-3 rel vs the 2e-2 gate):
  64KB per HW queue, landing ~0.5us earlier than bf16.
- a dummy 1x1 sqrt right after the DMA issues forces the sqrt act-table
  (set 3, which also holds Square) to load under the input transfers;
  act-table loads are non-blocking descriptor issues.
- d = pred - truth on DVE with the row sums fused in via accum_out.
- dsq = (32d)^2 runs on the otherwise-idle Scalar engine in two column
  halves so comb/mm pipeline behind it.
- comb = d*hsm2 + dsq on DVE (hsm2 scalar read straight from PSUM),
  main matmul in bf16 single pass, both in column halves.
- S2 matmul in bf16 (acc converted once) so the sqrt bias is ready
  before the second main matmul finishes.

Every core computes the full replicated result (inputs are tiny, far
below the ~20us collective floor, so replication beats batch-sharding +
AllReduce); core 0's scalar is returned.
"""

import numpy as np

_CACHE = {}


def _build_consts():
    # bf16 [128, 152]:
    #   cols 0:128   hconst[k,m] = -2 if k//2==m//2   (lhsT, pair-sum matmul)
    #   cols 128:136 mask8a[p,m] = 1 if m<4 and p%4==m    (lhsT, main mm half a)
    #   cols 136:144 mask8b[p,m] = 1 if m>=4 and p%4==m-4 (lhsT, main mm half b)
    #   cols 144:152 maskS8[p,m] = 2^-22 if (p%4)//2==(m%4)//2 (lhsT, S2 matmul)
    # The main matmul writes an [8, 256] PSUM tile (rows 0:4 = column half a,
    # rows 4:8 = half b) so the final Sqrt runs 256 long on 8 partitions.
    import ml_dtypes

    cb = np.zeros((128, 152), dtype=np.float32)
    k = np.arange(128)
    for m in range(128):
        cb[k[k // 2 == m // 2], m] = -2.0
    for m in range(4):
        cb[k[k % 4 == m], 128 + m] = 1.0
        cb[k[k % 4 == m], 140 + m] = 1.0
    for m in range(8):
        cb[k[(k % 4) // 2 == (m % 4) // 2], 144 + m] = 1.0 / 1024.0 / 4096.0
    return cb.astype(ml_dtypes.bfloat16)


def _build_nc():
    import concourse.tile as tile
    from concourse import bacc, mybir

    f32 = mybir.dt.float32
    bf16 = mybir.dt.bfloat16
    fp8 = mybir.dt.float8e4
    nc = bacc.Bacc("TRN2", target_bir_lowering=False, debug=False)
    in0 = nc.dram_tensor("in0", [128, 512], fp8, kind="ExternalInput").ap()
    in1 = nc.dram_tensor("in1", [128, 512], fp8, kind="ExternalInput").ap()
    cb = nc.dram_tensor("cb", [128, 152], bf16, kind="ExternalInput").ap()
    out = nc.dram_tensor("out", [1, 1], f32, kind="ExternalOutput").ap()

    H = 256

    with tile.TileContext(nc) as tc:
        with (
            tc.tile_pool(name="sb", bufs=1) as sb,
            tc.tile_pool(name="ps", bufs=1, space="PSUM") as ps,
        ):
            tin0 = sb.tile([128, 512], fp8, tag="tin0")
            tin1 = sb.tile([128, 512], fp8, tag="tin1")
            tcb = sb.tile([128, 152], bf16, tag="tcb")
            nc.sync.dma_start(tin0[:, :], in0)
            nc.scalar.dma_start(tin1[:, :], in1)
            nc.sync.dma_start(tcb[:, :], cb)
            hconst = tcb[:, 0:128]
            mask8a = tcb[:, 128:136]
            mask8b = tcb[:, 136:144]
            maskS8 = tcb[:, 144:152]

            # Dummy sqrt on a 1x1 scratch: forces the sqrt act-table load
            # (set 3, which also holds Square) to issue right after the DMA
            # issues instead of gating the final Sqrt.
            scratch = sb.tile([1, 1], f32, tag="scratch")
            dummy = sb.tile([1, 1], f32, tag="dummy")
            with tc.high_priority():
                nc.gpsimd.memset(scratch[:, :], 1.0)
                nc.scalar.activation(
                    dummy[:, :], scratch[:, :],
                    mybir.ActivationFunctionType.Sqrt,
                )

            td = sb.tile([128, 512], bf16, tag="td")
            dsq = sb.tile([128, 512], bf16, tag="dsq")
            red0 = sb.tile([128, 1], f32, tag="red0")
            red1 = sb.tile([128, 1], f32, tag="red1")
            acc_a = sb.tile([128, 1], f32, tag="acc_a")
            acc_b = sb.tile([128, 1], f32, tag="acc_b")

            # d = pred - truth with the row sum fused in via accum_out
            nc.vector.scalar_tensor_tensor(
                out=td[:, 0:H], in0=tin0[:, 0:H], scalar=1.0,
                in1=tin0[:, H:512],
                op0=mybir.AluOpType.mult, op1=mybir.AluOpType.subtract,
                accum_out=red0[:, :],
            )
            nc.vector.scalar_tensor_tensor(
                out=td[:, H:512], in0=tin1[:, 0:H], scalar=1.0,
                in1=tin1[:, H:512],
                op0=mybir.AluOpType.mult, op1=mybir.AluOpType.subtract,
                accum_out=red1[:, :],
            )
            # cs_db = red0 + red1 (bf16) unblocks the PE pair-sum matmul
            cs_db = sb.tile([128, 1], bf16, tag="cs_db")
            with tc.high_priority():
                nc.vector.tensor_add(cs_db[:, :], red0[:, :], red1[:, :])

            # hsm2[p] = -2*(cs_d[p] + cs_d[p^1])  (bf16 single-pass matmul)
            hsm2 = ps.tile([128, 1], f32, tag="hsm2")
            nc.tensor.matmul(hsm2[:, :], hconst, cs_db[:, :], start=True, stop=True)

            # dsq = (32*d)^2 = 1024*d^2 on the Scalar engine, in halves so
            # comb/mm pipeline behind it; acc_* catch the per-half row sums
            nc.scalar.activation(
                dsq[:, 0:H], td[:, 0:H], mybir.ActivationFunctionType.Square,
                scale=32.0, accum_out=acc_a[:, :],
            )
            nc.scalar.activation(
                dsq[:, H:512], td[:, H:512],
                mybir.ActivationFunctionType.Square,
                scale=32.0, accum_out=acc_b[:, :],
            )

            # comb = d*hsm2 + 1024*d^2; hsm2 scalar read straight from PSUM.
            # The two column halves land on PSUM rows 0:4 / 4:8 of one [8,256]
            # accumulation group so the final Sqrt is 256 long on 8 partitions.
            comb = sb.tile([128, 512], bf16, tag="comb")
            main = ps.tile([8, H], f32, tag="main")
            nc.vector.scalar_tensor_tensor(
                out=comb[:, 0:H], in0=td[:, 0:H], scalar=hsm2[:, :],
                in1=dsq[:, 0:H],
                op0=mybir.AluOpType.mult, op1=mybir.AluOpType.add,
            )
            nc.tensor.matmul(main[:, :], mask8a, comb[:, 0:H], start=True, stop=False)
            # acc in bf16 so the S2 matmul is a single bf16 pass; emitted
            # before comb_b so s2/bias are ready ahead of the last matmul
            acc_bf = sb.tile([128, 1], bf16, tag="acc_bf")
            nc.vector.tensor_add(acc_bf[:, :], acc_a[:, :], acc_b[:, :])
            s2 = ps.tile([8, 1], f32, tag="s2")
            nc.tensor.matmul(s2[:, :], maskS8, acc_bf[:, :], start=True, stop=True)
            bias = sb.tile([8, 1], f32, tag="bias")
            nc.vector.tensor_copy(bias[:, :], s2[:, :])
            nc.vector.scalar_tensor_tensor(
                out=comb[:, H:512], in0=td[:, H:512], scalar=hsm2[:, :],
                in1=dsq[:, H:512],
                op0=mybir.AluOpType.mult, op1=mybir.AluOpType.add,
            )
            nc.tensor.matmul(main[:, :], mask8b, comb[:, H:512], start=False, stop=True)

            # dist = sqrt(main/4096 + bias); dsums[m] = sum_j dist[m,j]
            # scale=2^-12 folds the /64 into the sqrt: sqrt(x/4096)=sqrt(x)/64
            dist = sb.tile([8, H], f32, tag="dist")
            dsums = sb.tile([8, 1], f32, tag="dsums")
            nc.scalar.activation(
                dist[:, :], main[:, :], mybir.ActivationFunctionType.Sqrt,
                bias=bias[:, :], scale=1.0 / 4096.0, accum_out=dsums[:, :],
            )

            # total = sum_m dsums[m]  (8-partition sum on gpsimd)
            out_sb = sb.tile([1, 1], f32, tag="out_sb")
            nc.gpsimd.tensor_reduce(
                out=out_sb[:, :], in_=dsums[:, :],
                axis=mybir.AxisListType.C, op=mybir.AluOpType.add,
            )
            nc.sync.dma_start(out, out_sb[:, :])

    nc.compile()
    return nc


def _get():
    if "nc" not in _CACHE:
        _CACHE["nc"] = _build_nc()
        _CACHE["cb"] = _build_consts()
    return _CACHE["nc"]


def _in_map(pred, truth):
    import ml_dtypes

    nc = _get()
    fp8 = ml_dtypes.float8_e4m3
    p = np.ascontiguousarray(np.asarray(pred, dtype=np.float32)).reshape(128, 512)
    t = np.ascontiguousarray(np.asarray(truth, dtype=np.float32)).reshape(128, 512)
    H = 256
    in0 = np.empty((128, 512), dtype=fp8)
    in1 = np.empty((128, 512), dtype=fp8)
    in0[:, 0:H] = p[:, 0:H].astype(fp8)
    in0[:, H:512] = t[:, 0:H].astype(fp8)
    in1[:, 0:H] = p[:, H:512].astype(fp8)
    in1[:, H:512] = t[:, H:512].astype(fp8)
    return nc, {"in0": in0, "in1": in1, "cb": _CACHE["cb"]}


def kernel(pred, truth) -> np.ndarray:
    from concourse.bass_utils import run_bass_kernel_spmd

    nc, in_map = _in_map(pred, truth)
    res = run_bass_kernel_spmd(
        nc, [dict(in_map) for _ in range(8)], core_ids=list(range(8))
    )
    return res.results[0]["out"].reshape(()).astype(np.float32)


# revision 17
# speedup vs baseline: 1.2520x; 1.0220x over previous
"""Gcs pairwise-distance loss kernel for Trainium2 (Bass/Tile), 8-core SPMD.

Math: with d = pred - truth, dX = d[:, :P], dY = d[:, P:] (B=32, P=1024),
    sumsq_h[i] = sum_{b,j} (v[b,j] - v[b,i])^2
               = S2_h + sum_b (1024*v[b,i]^2 - 2*rs_h[b]*v[b,i])
where rs_h[b] = sum_j v[b,j], S2_h = sum_{b,j} v[b,j]^2.  The loss is
    (sum_i sqrt(sumsq_X[i]) + sum_i sqrt(sumsq_Y[i])) / 64.
This collapses the O(B*P^2) pairwise reduction to O(B*P).

Layout: d [32, 2048] is viewed as [128, 512]; partition p = 4*b + c where
c in {0,1} covers X columns and {2,3} covers Y columns.

v4 schedule (traced against v2/v3 on neuron-profile):
- inputs are cast to fp8-e4m3 on the host (loss is a 65k-term reduction:
  quantization error averages out, measured ~1e-3 rel vs the 2e-2 gate):
  64KB per HW queue, landing ~0.5us earlier than bf16.
- a dummy 1x1 sqrt right after the DMA issues forces the sqrt act-table
  (set 3, which also holds Square) to load under the input transfers;
  act-table loads are non-blocking descriptor issues.
- d = pred - truth on DVE with the row sums fused in via accum_out.
- dsq = (32d)^2 runs on the otherwise-idle Scalar engine in two column
  halves so comb/mm pipeline behind it.
- comb = d*hsm2 + dsq on DVE (hsm2 scalar read straight from PSUM),
  main matmul in bf16 single pass, both in column halves.
- S2 matmul in bf16 (acc converted once) so the sqrt bias is ready
  before the second main matmul finishes.

Every core computes the full replicated result (inputs are tiny, far
below the ~20us collective floor, so replication beats batch-sharding +
AllReduce); core 0's scalar is returned.
"""

import numpy as np

_CACHE = {}


def _build_consts():
    # bf16 [128, 152]:
    #   cols 0:128   hconst[k,m] = -2 if k//2==m//2   (lhsT, pair-sum matmul)
    #   cols 128:136 mask8a[p,m] = 1 if m<4 and p%4==m    (lhsT, main mm half a)
    #   cols 136:144 mask8b[p,m] = 1 if m>=4 and p%4==m-4 (lhsT, main mm half b)
    #   cols 144:152 maskS8[p,m] = 2^-22 if (p%4)//2==(m%4)//2 (lhsT, S2 matmul)
    # The main matmul writes an [8, 256] PSUM tile (rows 0:4 = column half a,
    # rows 4:8 = half b) so the final Sqrt runs 256 long on 8 partitions.
    import ml_dtypes

    cb = np.zeros((128, 152), dtype=np.float32)
    k = np.arange(128)
    for m in range(128):
        cb[k[k // 2 == m // 2], m] = -2.0
    for m in range(4):
        cb[k[k % 4 == m], 128 + m] = 1.0
        cb[k[k % 4 == m], 140 + m] = 1.0
    for m in range(8):
        cb[k[(k % 4) // 2 == (m % 4) // 2], 144 + m] = 1.0 / 1024.0 / 4096.0
    return cb.astype(ml_dtypes.bfloat16)


def _build_nc():
    import concourse.tile as tile
    from concourse import bacc, mybir

    f32 = mybir.dt.float32
    bf16 = mybir.dt.bfloat16
    fp8 = mybir.dt.float8e4
    nc = bacc.Bacc("TRN2", target_bir_lowering=False, debug=False)
    in0 = nc.dram_tensor("in0", [128, 512], fp8, kind="ExternalInput").ap()
    in1 = nc.dram_tensor("in1", [128, 512], fp8, kind="ExternalInput").ap()
    cb = nc.dram_tensor("cb", [128, 152], bf16, kind="ExternalInput").ap()
    out = nc.dram_tensor("out", [1, 1], f32, kind="ExternalOutput").ap()
    # raw (non-tile) SBUF home for the final scalar + a dedicated DMA
    # semaphore: the out DMA is issued after the tile-context exit barrier
    # with its completion sem attached via then_inc (walrus requires one on
    # dynamic DMAs), so the tile epilogue never stalls on the 4-byte flight;
    # the NEFF teardown's queue drains cover it.
    out_home = nc.alloc_sbuf_tensor("out_home", [1, 1], f32)
    out_sem = nc.alloc_semaphore("out_dma_sem")

    H = 256

    with tile.TileContext(nc) as tc:
        with (
            tc.tile_pool(name="sb", bufs=1) as sb,
            tc.tile_pool(name="ps", bufs=1, space="PSUM") as ps,
        ):
            tin0 = sb.tile([128, 512], fp8, tag="tin0")
            tin1 = sb.tile([128, 512], fp8, tag="tin1")
            tcb = sb.tile([128, 152], bf16, tag="tcb")
            nc.sync.dma_start(tin0[:, :], in0)
            nc.scalar.dma_start(tin1[:, :], in1)
            nc.sync.dma_start(tcb[:, :], cb)
            hconst = tcb[:, 0:128]
            mask8a = tcb[:, 128:136]
            mask8b = tcb[:, 136:144]
            maskS8 = tcb[:, 144:152]

            # Dummy sqrt on a 1x1 scratch: forces the sqrt act-table load
            # (set 3, which also holds Square) to issue right after the DMA
            # issues instead of gating the final Sqrt.
            scratch = sb.tile([1, 1], f32, tag="scratch")
            dummy = sb.tile([1, 1], f32, tag="dummy")
            with tc.high_priority():
                nc.gpsimd.memset(scratch[:, :], 1.0)
                nc.scalar.activation(
                    dummy[:, :], scratch[:, :],
                    mybir.ActivationFunctionType.Sqrt,
                )

            td = sb.tile([128, 512], bf16, tag="td")
            dsq = sb.tile([128, 512], bf16, tag="dsq")
            red0 = sb.tile([128, 1], f32, tag="red0")
            red1 = sb.tile([128, 1], f32, tag="red1")
            acc_a = sb.tile([128, 1], f32, tag="acc_a")
            acc_b = sb.tile([128, 1], f32, tag="acc_b")

            # d = pred - truth with the row sum fused in via accum_out
            nc.vector.scalar_tensor_tensor(
                out=td[:, 0:H], in0=tin0[:, 0:H], scalar=1.0,
                in1=tin0[:, H:512],
                op0=mybir.AluOpType.mult, op1=mybir.AluOpType.subtract,
                accum_out=red0[:, :],
            )
            nc.vector.scalar_tensor_tensor(
                out=td[:, H:512], in0=tin1[:, 0:H], scalar=1.0,
                in1=tin1[:, H:512],
                op0=mybir.AluOpType.mult, op1=mybir.AluOpType.subtract,
                accum_out=red1[:, :],
            )
            # cs_db = red0 + red1 (bf16) unblocks the PE pair-sum matmul
            cs_db = sb.tile([128, 1], bf16, tag="cs_db")
            with tc.high_priority():
                nc.vector.tensor_add(cs_db[:, :], red0[:, :], red1[:, :])

            # hsm2[p] = -2*(cs_d[p] + cs_d[p^1])  (bf16 single-pass matmul)
            hsm2 = ps.tile([128, 1], f32, tag="hsm2")
            nc.tensor.matmul(hsm2[:, :], hconst, cs_db[:, :], start=True, stop=True)

            # dsq = (32*d)^2 = 1024*d^2 on the Scalar engine, in halves so
            # comb/mm pipeline behind it; acc_* catch the per-half row sums
            nc.scalar.activation(
                dsq[:, 0:H], td[:, 0:H], mybir.ActivationFunctionType.Square,
                scale=32.0, accum_out=acc_a[:, :],
            )
            nc.scalar.activation(
                dsq[:, H:512], td[:, H:512],
                mybir.ActivationFunctionType.Square,
                scale=32.0, accum_out=acc_b[:, :],
            )

            # comb = d*hsm2 + 1024*d^2; hsm2 scalar read straight from PSUM.
            # The two column halves land on PSUM rows 0:4 / 4:8 of one [8,256]
            # accumulation group so the final Sqrt is 256 long on 8 partitions.
            comb = sb.tile([128, 512], bf16, tag="comb")
            main = ps.tile([8, H], f32, tag="main")
            nc.vector.scalar_tensor_tensor(
                out=comb[:, 0:H], in0=td[:, 0:H], scalar=hsm2[:, :],
                in1=dsq[:, 0:H],
                op0=mybir.AluOpType.mult, op1=mybir.AluOpType.add,
            )
            nc.tensor.matmul(main[:, :], mask8a, comb[:, 0:H], start=True, stop=False)
            # acc in bf16 so the S2 matmul is a single bf16 pass; emitted
            # before comb_b so s2/bias are ready ahead of the last matmul
            acc_bf = sb.tile([128, 1], bf16, tag="acc_bf")
            nc.vector.tensor_add(acc_bf[:, :], acc_a[:, :], acc_b[:, :])
            s2 = ps.tile([8, 1], f32, tag="s2")
            nc.tensor.matmul(s2[:, :], maskS8, acc_bf[:, :], start=True, stop=True)
            bias = sb.tile([8, 1], f32, tag="bias")
            nc.vector.tensor_copy(bias[:, :], s2[:, :])
            nc.vector.scalar_tensor_tensor(
                out=comb[:, H:512], in0=td[:, H:512], scalar=hsm2[:, :],
                in1=dsq[:, H:512],
                op0=mybir.AluOpType.mult, op1=mybir.AluOpType.add,
            )
            nc.tensor.matmul(main[:, :], mask8b, comb[:, H:512], start=False, stop=True)

            # dist = sqrt(main/4096 + bias); dsums[m] = sum_j dist[m,j]
            # scale=2^-12 folds the /64 into the sqrt: sqrt(x/4096)=sqrt(x)/64
            dist = sb.tile([8, H], f32, tag="dist")
            dsums = sb.tile([8, 1], f32, tag="dsums")
            nc.scalar.activation(
                dist[:, :], main[:, :], mybir.ActivationFunctionType.Sqrt,
                bias=bias[:, :], scale=1.0 / 4096.0, accum_out=dsums[:, :],
            )

            # total = sum_m dsums[m]  (8-partition sum on gpsimd)
            nc.gpsimd.tensor_reduce(
                out=out_home.ap(), in_=dsums[:, :],
                axis=mybir.AxisListType.C, op=mybir.AluOpType.add,
            )

    # post-exit-barrier fire-and-forget output DMA (see out_home above)
    nc.sync.dma_start(out, out_home.ap()).then_inc(out_sem, 16)

    nc.compile()
    return nc


def _get():
    if "nc" not in _CACHE:
        _CACHE["nc"] = _build_nc()
        _CACHE["cb"] = _build_consts()
    return _CACHE["nc"]


def _in_map(pred, truth):
    import ml_dtypes

    nc = _get()
    fp8 = ml_dtypes.float8_e4m3
    p = np.ascontiguousarray(np.asarray(pred, dtype=np.float32)).reshape(128, 512)
    t = np.ascontiguousarray(np.asarray(truth, dtype=np.float32)).reshape(128, 512)
    H = 256
    in0 = np.empty((128, 512), dtype=fp8)
    in1 = np.empty((128, 512), dtype=fp8)
    in0[:, 0:H] = p[:, 0:H].astype(fp8)
    in0[:, H:512] = t[:, 0:H].astype(fp8)
    in1[:, 0:H] = p[:, H:512].astype(fp8)
    in1[:, H:512] = t[:, H:512].astype(fp8)
    return nc, {"in0": in0, "in1": in1, "cb": _CACHE["cb"]}


def kernel(pred, truth) -> np.ndarray:
    from concourse.bass_utils import run_bass_kernel_spmd

    nc, in_map = _in_map(pred, truth)
    res = run_bass_kernel_spmd(
        nc, [dict(in_map) for _ in range(8)], core_ids=list(range(8))
    )
    return res.results[0]["out"].reshape(()).astype(np.float32)


# revision 19
# speedup vs baseline: 1.2845x; 1.0260x over previous
"""Gcs pairwise-distance loss kernel for Trainium2 (Bass/Tile), 8-core SPMD.

Math: with d = pred - truth, dX = d[:, :P], dY = d[:, P:] (B=32, P=1024),
    sumsq_h[i] = sum_{b,j} (v[b,j] - v[b,i])^2
               = S2_h + sum_b (1024*v[b,i]^2 - 2*rs_h[b]*v[b,i])
where rs_h[b] = sum_j v[b,j], S2_h = sum_{b,j} v[b,j]^2.  The loss is
    (sum_i sqrt(sumsq_X[i]) + sum_i sqrt(sumsq_Y[i])) / 64.
This collapses the O(B*P^2) pairwise reduction to O(B*P).

Layout: d [32, 2048] is viewed as [128, 512]; partition p = 4*b + c where
c in {0,1} covers X columns and {2,3} covers Y columns.

Schedule (iterated against neuron-profile traces; measured fixed costs:
~12.3us framework floor incl. a ~7us NEFF-teardown semaphore storm,
~900ns DMA sem-prop, ~360ns ACT->DVE handoff):
- inputs are cast to fp8-e4m3 on the host (loss is a 65k-term reduction:
  quantization error averages out, measured 2.8e-4 rel vs the 2e-2 gate):
  64KB per HW queue.
- a dummy 1x1 sqrt right after the DMA issues forces the sqrt act-table
  (set 3, which also holds Square) to load under the input transfers;
  act-table loads are non-blocking descriptor issues.
- d = pred - truth on DVE with the row sums fused in via accum_out
  (scalar_tensor_tensor has a free accumulator; it never runs 2x, so
  fp8 inputs cost nothing on DVE).
- dsq = (32d)^2 runs on the otherwise-idle Scalar engine in two column
  halves so square_a -> comb_a -> mm_a pipelines while square_b runs.
- comb = d*hsm2 + dsq on DVE (hsm2 scalar read straight from PSUM);
  the two bf16 single-pass main matmuls write rows 0:4 / 4:8 of one
  [8, 256] PSUM group so the final Sqrt runs 256 long on 8 partitions.
- S2 matmul in bf16 (acc converted once), emitted before comb_b so the
  sqrt bias lands before the last main matmul does.
- the 4-byte output DMA is issued after the tile-context exit barrier
  with its completion sem attached via then_inc, so the tile epilogue
  never waits out the DMA flight; the NEFF teardown drains cover it.

Every core computes the full replicated result (inputs are tiny, far
below the ~20us collective floor, so replication beats batch-sharding +
AllReduce); core 0's scalar is returned.
"""

import numpy as np

_CACHE = {}


def _build_consts():
    # bf16 [128, 152]:
    #   cols 0:128   hconst[k,m] = -2 if k//2==m//2   (lhsT, pair-sum matmul)
    #   cols 128:136 mask8a[p,m] = 1 if m<4 and p%4==m    (lhsT, main mm half a)
    #   cols 136:144 mask8b[p,m] = 1 if m>=4 and p%4==m-4 (lhsT, main mm half b)
    #   cols 144:152 maskS8[p,m] = 2^-22 if (p%4)//2==(m%4)//2 (lhsT, S2 matmul)
    # The main matmul writes an [8, 256] PSUM tile (rows 0:4 = column half a,
    # rows 4:8 = half b) so the final Sqrt runs 256 long on 8 partitions.
    import ml_dtypes

    cb = np.zeros((128, 152), dtype=np.float32)
    k = np.arange(128)
    for m in range(128):
        cb[k[k // 2 == m // 2], m] = -2.0
    for m in range(4):
        cb[k[k % 4 == m], 128 + m] = 1.0
        cb[k[k % 4 == m], 140 + m] = 1.0
    for m in range(8):
        cb[k[(k % 4) // 2 == (m % 4) // 2], 144 + m] = 1.0 / 1024.0 / 4096.0
    return cb.astype(ml_dtypes.bfloat16)


def _build_nc():
    import concourse.tile as tile
    from concourse import bacc, mybir

    f32 = mybir.dt.float32
    bf16 = mybir.dt.bfloat16
    fp8 = mybir.dt.float8e4
    nc = bacc.Bacc("TRN2", target_bir_lowering=False, debug=False)
    in0 = nc.dram_tensor("in0", [128, 512], fp8, kind="ExternalInput").ap()
    in1 = nc.dram_tensor("in1", [128, 512], fp8, kind="ExternalInput").ap()
    cb = nc.dram_tensor("cb", [128, 152], bf16, kind="ExternalInput").ap()
    out = nc.dram_tensor("out", [1, 1], f32, kind="ExternalOutput").ap()
    # raw (non-tile) SBUF home for the final scalar + a dedicated DMA
    # semaphore: the out DMA is issued after the tile-context exit barrier
    # with its completion sem attached via then_inc (walrus requires one on
    # dynamic DMAs), so the tile epilogue never stalls on the 4-byte flight;
    # the NEFF teardown's queue drains cover it.
    out_home = nc.alloc_sbuf_tensor("out_home", [1, 1], f32)
    out_sem = nc.alloc_semaphore("out_dma_sem")

    H = 256

    with tile.TileContext(nc) as tc:
        with (
            tc.tile_pool(name="sb", bufs=1) as sb,
            tc.tile_pool(name="ps", bufs=1, space="PSUM") as ps,
        ):
            tin0 = sb.tile([128, 512], fp8, tag="tin0")
            tin1 = sb.tile([128, 512], fp8, tag="tin1")
            tcb = sb.tile([128, 152], bf16, tag="tcb")
            nc.sync.dma_start(tin0[:, :], in0)
            nc.scalar.dma_start(tin1[:, :], in1)
            nc.sync.dma_start(tcb[:, :], cb)
            hconst = tcb[:, 0:128]
            mask8a = tcb[:, 128:136]
            mask8b = tcb[:, 136:144]
            maskS8 = tcb[:, 144:152]

            # Dummy sqrt on a 1x1 scratch: forces the sqrt act-table load
            # (set 3, which also holds Square) to issue right after the DMA
            # issues instead of gating the final Sqrt.
            scratch = sb.tile([1, 1], f32, tag="scratch")
            dummy = sb.tile([1, 1], f32, tag="dummy")
            with tc.high_priority():
                nc.gpsimd.memset(scratch[:, :], 1.0)
                nc.scalar.activation(
                    dummy[:, :], scratch[:, :],
                    mybir.ActivationFunctionType.Sqrt,
                )

            td = sb.tile([128, 512], bf16, tag="td")
            dsq = sb.tile([128, 512], bf16, tag="dsq")
            red0 = sb.tile([128, 1], f32, tag="red0")
            red1 = sb.tile([128, 1], f32, tag="red1")
            acc_a = sb.tile([128, 1], f32, tag="acc_a")
            acc_b = sb.tile([128, 1], f32, tag="acc_b")

            # d = pred - truth with the row sum fused in via accum_out
            nc.vector.scalar_tensor_tensor(
                out=td[:, 0:H], in0=tin0[:, 0:H], scalar=1.0,
                in1=tin0[:, H:512],
                op0=mybir.AluOpType.mult, op1=mybir.AluOpType.subtract,
                accum_out=red0[:, :],
            )
            nc.vector.scalar_tensor_tensor(
                out=td[:, H:512], in0=tin1[:, 0:H], scalar=1.0,
                in1=tin1[:, H:512],
                op0=mybir.AluOpType.mult, op1=mybir.AluOpType.subtract,
                accum_out=red1[:, :],
            )
            # cs_db = red0 + red1 (bf16) unblocks the PE pair-sum matmul
            cs_db = sb.tile([128, 1], bf16, tag="cs_db")
            with tc.high_priority():
                nc.vector.tensor_add(cs_db[:, :], red0[:, :], red1[:, :])

            # hsm2[p] = -2*(cs_d[p] + cs_d[p^1])  (bf16 single-pass matmul)
            hsm2 = ps.tile([128, 1], f32, tag="hsm2")
            nc.tensor.matmul(hsm2[:, :], hconst, cs_db[:, :], start=True, stop=True)

            # dsq = (32*d)^2 = 1024*d^2 on the Scalar engine, in halves so
            # comb/mm pipeline behind it; acc_* catch the per-half row sums
            nc.scalar.activation(
                dsq[:, 0:H], td[:, 0:H], mybir.ActivationFunctionType.Square,
                scale=32.0, accum_out=acc_a[:, :],
            )
            nc.scalar.activation(
                dsq[:, H:512], td[:, H:512],
                mybir.ActivationFunctionType.Square,
                scale=32.0, accum_out=acc_b[:, :],
            )

            # comb = d*hsm2 + 1024*d^2; hsm2 scalar read straight from PSUM.
            # The two column halves land on PSUM rows 0:4 / 4:8 of one [8,256]
            # accumulation group so the final Sqrt is 256 long on 8 partitions.
            comb = sb.tile([128, 512], bf16, tag="comb")
            main = ps.tile([8, H], f32, tag="main")
            nc.vector.scalar_tensor_tensor(
                out=comb[:, 0:H], in0=td[:, 0:H], scalar=hsm2[:, :],
                in1=dsq[:, 0:H],
                op0=mybir.AluOpType.mult, op1=mybir.AluOpType.add,
            )
            nc.tensor.matmul(main[:, :], mask8a, comb[:, 0:H], start=True, stop=False)
            # acc in bf16 so the S2 matmul is a single bf16 pass; emitted
            # before comb_b so s2/bias are ready ahead of the last matmul
            acc_bf = sb.tile([128, 1], bf16, tag="acc_bf")
            nc.vector.tensor_add(acc_bf[:, :], acc_a[:, :], acc_b[:, :])
            s2 = ps.tile([8, 1], f32, tag="s2")
            nc.tensor.matmul(s2[:, :], maskS8, acc_bf[:, :], start=True, stop=True)
            bias = sb.tile([8, 1], f32, tag="bias")
            nc.vector.tensor_copy(bias[:, :], s2[:, :])
            nc.vector.scalar_tensor_tensor(
                out=comb[:, H:512], in0=td[:, H:512], scalar=hsm2[:, :],
                in1=dsq[:, H:512],
                op0=mybir.AluOpType.mult, op1=mybir.AluOpType.add,
            )
            nc.tensor.matmul(main[:, :], mask8b, comb[:, H:512], start=False, stop=True)

            # dist = sqrt(main/4096 + bias); dsums[m] = sum_j dist[m,j]
            # scale=2^-12 folds the /64 into the sqrt: sqrt(x/4096)=sqrt(x)/64
            dist = sb.tile([8, H], f32, tag="dist")
            dsums = sb.tile([8, 1], f32, tag="dsums")
            nc.scalar.activation(
                dist[:, :], main[:, :], mybir.ActivationFunctionType.Sqrt,
                bias=bias[:, :], scale=1.0 / 4096.0, accum_out=dsums[:, :],
            )

            # total = sum_m dsums[m]  (8-partition sum on gpsimd)
            nc.gpsimd.tensor_reduce(
                out=out_home.ap(), in_=dsums[:, :],
                axis=mybir.AxisListType.C, op=mybir.AluOpType.add,
            )

    # post-exit-barrier fire-and-forget output DMA (see out_home above)
    nc.sync.dma_start(out, out_home.ap(), single_packet=True).then_inc(out_sem, 16)

    nc.compile()
    return nc


def _get():
    if "nc" not in _CACHE:
        _CACHE["nc"] = _build_nc()
        _CACHE["cb"] = _build_consts()
    return _CACHE["nc"]


def _in_map(pred, truth):
    import ml_dtypes

    nc = _get()
    fp8 = ml_dtypes.float8_e4m3
    p = np.ascontiguousarray(np.asarray(pred, dtype=np.float32)).reshape(128, 512)
    t = np.ascontiguousarray(np.asarray(truth, dtype=np.float32)).reshape(128, 512)
    H = 256
    in0 = np.empty((128, 512), dtype=fp8)
    in1 = np.empty((128, 512), dtype=fp8)
    in0[:, 0:H] = p[:, 0:H].astype(fp8)
    in0[:, H:512] = t[:, 0:H].astype(fp8)
    in1[:, 0:H] = p[:, H:512].astype(fp8)
    in1[:, H:512] = t[:, H:512].astype(fp8)
    return nc, {"in0": in0, "in1": in1, "cb": _CACHE["cb"]}


def kernel(pred, truth) -> np.ndarray:
    from concourse.bass_utils import run_bass_kernel_spmd

    nc, in_map = _in_map(pred, truth)
    res = run_bass_kernel_spmd(
        nc, [dict(in_map) for _ in range(8)], core_ids=list(range(8))
    )
    return res.results[0]["out"].reshape(()).astype(np.float32)
